# revision 1
# baseline (speedup 1.0000x reference)
"""Trainium2 Bass kernel for the lifted-structure metric loss (nn_Metric_Loss).

Math (reference): for X in {T (text), Z (interleaved text/shape)}:
    D = X @ X.T                      [4096, 4096]
    E = exp(0.5 + D)
    per pair p (rows i=2p, j=2p+1): S[p] = sum(E[{i,j}, :]) - sum(E[{i,j},{i,j}])
    J[p] = relu(log(S[p]) - D[i,j])^2
    loss_X = mean(J) / 2;  total = loss_T + 2 * loss_Z

Sharding (symmetric): E is symmetric, so only the 36 upper-triangle
[512,512] blocks per loss are computed. 72 block-tasks (both losses) are
dealt 9 per core; the host gathers each task's lhs/rhs column blocks into a
per-core input tensor, so the SPMD program is identical across cores and a
slot doesn't know (or care) which loss/block it computes. Per slot the
device emits: ACT-accumulated row sums of E (rows of block i), a PE
ones-vector col-sum of E (rows of block j, used when i != j), and the
2x2-pair-block corrections + positive-pair sims (used when i == j).
The host does the final O(N) assembly + log/relu/square/mean in float64.

Matmuls run in fp8 e4m3 with DoubleRow (2 MACs/cell/cycle); end-to-end
relative error vs the fp32 reference is ~1e-5.
"""

import numpy as np
import ml_dtypes

import concourse.mybir as mybir
import concourse.tile as tile
from concourse import bacc
from concourse.bass import ds
from concourse.bass_utils import run_bass_kernel_spmd

N, D_EMB = 4096, 1024
P_PAIRS = N // 2
NCORES = 8
B = 512                   # block size
NB = N // B               # 8x8 block grid
MT = B // 128             # 4 m-tiles per block
KC = D_EMB // 128         # 8 k-chunks
NSLOTS = 9                # tasks per core
MARGIN = 0.5

# fixed task deal: 36 upper-tri blocks x 2 losses -> 9 per core. Diagonal
# tasks (2 per core) sit at slots 6-7 so their serial DVE mask chains overlap
# slot 8's matmuls; the final slot is off-diagonal, whose tail chain (exp ->
# two DVE adds -> DMA) is the shortest. Off-diag slots: 0-5 and 8.
NOFF = 7
DIAG_SLOTS = (6, 7)
ACC_IDX = {0: 0, 1: 1, 2: 2, 3: 3, 4: 4, 5: 5, 8: 6}  # off-diag slot -> acc row
_DIAG = [(l, i, i) for l in range(2) for i in range(NB)]
_OFF = [(l, i, j) for l in range(2) for i in range(NB) for j in range(i + 1, NB)]
SLOTS = [
    _OFF[7 * c : 7 * c + 6] + _DIAG[2 * c : 2 * (c + 1)] + [_OFF[7 * c + 6]]
    for c in range(NCORES)
]

_CACHE = {}


def _build_nc():
    nc = bacc.Bacc(
        "TRN2",
        target_bir_lowering=False,
        debug=False,
        num_devices=NCORES,
        enable_partition_id=False,
        monotonic_sem_count=0,
    )
    f32 = mybir.dt.float32
    bf16 = mybir.dt.bfloat16
    fp8 = mybir.dt.float8e4
    blk = nc.dram_tensor(
        "blk", [NSLOTS, 2, 128, KC, B], fp8, kind="ExternalInput"
    ).ap()
    m2 = nc.dram_tensor("mask2", [128, 128], f32, kind="ExternalInput").ap()
    mij = nc.dram_tensor("maskij", [128, 128], f32, kind="ExternalInput").ap()
    # [128, 3*36]: rp | eb | dij column groups
    out_main = nc.dram_tensor(
        "out_main", [128, 3 * NSLOTS * MT], f32, kind="ExternalOutput"
    ).ap()
    # per off-diag slot: sum of the 4 exp tiles; host reduces partitions
    out_acc = nc.dram_tensor(
        "out_acc", [NOFF, 128, B], bf16, kind="ExternalOutput"
    ).ap()

    with tile.TileContext(nc) as tc:
        with (
            tc.tile_pool(name="xb", bufs=4) as xb_pool,
            tc.tile_pool(name="consts", bufs=1) as consts,
            tc.tile_pool(name="psum", bufs=7, space="PSUM") as psum_pool,
            tc.tile_pool(name="esc", bufs=4) as esc_pool,
            tc.tile_pool(name="stats", bufs=3) as stats,
        ):
            bias_sb = consts.tile([128, 1], f32, tag="bias")
            nc.vector.memset(bias_sb, MARGIN)
            main_sb = consts.tile([128, 3 * NSLOTS * MT], f32, tag="main")
            nc.vector.memset(main_sb, 0.0)
            NM = NSLOTS * MT
            rp_sb = main_sb[:, 0:NM]
            eb_sb = main_sb[:, NM : 2 * NM]
            dij_sb = main_sb[:, 2 * NM : 3 * NM]

            # per-slot input tiles; slot 0 is split by k-chunk pairs so the
            # first matmuls start as soon as their chunks land
            xbs = []
            for s in range(NSLOTS):
                xb = xb_pool.tile([128, 2, KC, B], fp8, tag="xb")
                if s == 0:
                    for p in range(KC // 2):
                        nc.sync.dma_start(
                            out=xb[:, :, 2 * p : 2 * p + 2, :],
                            in_=blk[s, :, :, 2 * p : 2 * p + 2, :].rearrange(
                                "two p kc c -> p two kc c"
                            ),
                        )
                elif s in (1, 2):
                    for p in range(2):
                        nc.sync.dma_start(
                            out=xb[:, :, 4 * p : 4 * p + 4, :],
                            in_=blk[s, :, :, 4 * p : 4 * p + 4, :].rearrange(
                                "two p kc c -> p two kc c"
                            ),
                        )
                else:
                    nc.sync.dma_start(
                        out=xb,
                        in_=blk[s].rearrange("two p kc c -> p two kc c"),
                    )
                xbs.append(xb)
            # masks load after the block data (not needed until the diag
            # slots at the end; keeps the head DMA queue clear)
            m2_sb = consts.tile([128, 128], f32, tag="m2")
            nc.sync.dma_start(out=m2_sb, in_=m2)
            mij_sb = consts.tile([128, 128], f32, tag="mij")
            nc.sync.dma_start(out=mij_sb, in_=mij)

            last_acc = None
            for s in range(NSLOTS):
                xb = xbs[s]
                esc = esc_pool.tile([128, MT, B], mybir.dt.float32, tag="esc")
                chain = None  # last slot: incremental add chain
                for t in range(MT):
                    col = s * MT + t
                    dpsum = psum_pool.tile([128, B], mybir.dt.float32, tag="dps")
                    for k2 in range(KC // 2):
                        nc.tensor.matmul(
                            dpsum,
                            xb[:, 0, 2 * k2 : 2 * k2 + 2, ds(128 * t, 128)],
                            xb[:, 1, 2 * k2 : 2 * k2 + 2, :],
                            start=(k2 == 0),
                            stop=(k2 == KC // 2 - 1),
                            perf_mode=mybir.MatmulPerfMode.DoubleRow,
                        )
                    nc.scalar.activation(
                        esc[:, t, :],
                        dpsum,
                        mybir.ActivationFunctionType.Exp,
                        bias=bias_sb,
                        scale=1.0,
                        accum_out=rp_sb[:, col : col + 1],
                    )
                    if s == NSLOTS - 1 and t > 0:
                        # last slot: chain the col-sum adds incrementally so
                        # only one DVE add trails the final exp
                        nxt = stats.tile([128, B],
                                         bf16 if t == MT - 1 else mybir.dt.float32,
                                         tag=f"chain{t}")
                        prev = esc[:, 0, :] if t == 1 else chain
                        nc.vector.tensor_add(nxt, prev, esc[:, t, :])
                        chain = nxt
                    if s in DIAG_SLOTS:
                        # pair-block corrections, diagonal slots only
                        mblk = stats.tile([128, 128], mybir.dt.float32, tag="mblk")
                        nc.vector.tensor_mul(
                            mblk, esc[:, t, ds(128 * t, 128)], m2_sb
                        )
                        nc.vector.reduce_sum(
                            out=eb_sb[:, col : col + 1],
                            in_=mblk,
                            axis=mybir.AxisListType.X,
                        )
                        mblk2 = stats.tile([128, 128], mybir.dt.float32, tag="mblk2")
                        nc.vector.tensor_mul(
                            mblk2, dpsum[:, ds(128 * t, 128)], mij_sb
                        )
                        nc.vector.reduce_sum(
                            out=dij_sb[:, col : col + 1],
                            in_=mblk2,
                            axis=mybir.AxisListType.X,
                        )
                if s == NSLOTS - 1:
                    last_acc = chain
                elif s not in DIAG_SLOTS:
                    # col-sum prep, off-diagonal slots: accumulate the 4 exp
                    # tiles (add tree split over DVE + idle GpSimd) and
                    # stream to DRAM; the host reduces over partitions
                    acc01 = stats.tile([128, B], mybir.dt.float32, tag="acc01")
                    nc.vector.tensor_add(acc01, esc[:, 0, :], esc[:, 1, :])
                    acc23 = stats.tile([128, B], mybir.dt.float32, tag="acc23")
                    nc.gpsimd.tensor_add(acc23, esc[:, 2, :], esc[:, 3, :])
                    acc_bf = stats.tile([128, B], bf16, tag="accbf")
                    nc.vector.tensor_add(acc_bf, acc01, acc23)
                    nc.sync.dma_start(out=out_acc[ACC_IDX[s]], in_=acc_bf)
            # out_main first: its setup overlaps the last slot's add chain
            nc.sync.dma_start(out=out_main, in_=main_sb)
            nc.sync.dma_start(out=out_acc[ACC_IDX[NSLOTS - 1]], in_=last_acc)
    nc.compile()
    return nc


def _get_nc():
    if "nc" not in _CACHE:
        _CACHE["nc"] = _build_nc()
    return _CACHE["nc"]


def _make_in_maps(text_embeddings, shape_embeddings):
    T = np.asarray(text_embeddings, dtype=np.float32)
    S = np.asarray(shape_embeddings, dtype=np.float32)
    Z = np.empty_like(T)
    Z[0::2] = T[0::2]
    Z[1::2] = S
    # [loss][128 p, KC, N] fp8: row-block p of X^T chunk kc, cols
    Xg = []
    for X in (T, Z):
        XT = np.ascontiguousarray(X.T).astype(ml_dtypes.float8_e4m3)
        Xg.append(XT.reshape(KC, 128, N).transpose(1, 0, 2))  # [128, KC, N]
    r = np.arange(128)
    mask2 = (r[:, None] // 2 == r[None, :] // 2).astype(np.float32)
    maskij = ((r[:, None] % 2 == 0) & (r[None, :] == r[:, None] + 1)).astype(
        np.float32
    )
    in_maps = []
    for c in range(NCORES):
        blk = np.empty((NSLOTS, 2, 128, KC, B), dtype=ml_dtypes.float8_e4m3)
        for s, (l, i, j) in enumerate(SLOTS[c]):
            blk[s, 0] = Xg[l][:, :, B * i : B * (i + 1)]
            blk[s, 1] = Xg[l][:, :, B * j : B * (j + 1)]
        in_maps.append({"blk": blk, "mask2": mask2, "maskij": maskij})
    return in_maps


def _finalize(outs):
    """outs: list of 8 per-core output dicts -> scalar loss."""
    row_s = [np.zeros(N, np.float64) for _ in range(2)]
    dij_all = [np.zeros(N, np.float64) for _ in range(2)]
    nm = NSLOTS * MT
    for c, o in enumerate(outs):
        main = np.asarray(o["out_main"], np.float64)
        rp = main[:, 0:nm]
        eb = main[:, nm : 2 * nm]
        dj = main[:, 2 * nm : 3 * nm]
        # col sums: reduce the shipped per-slot exp-sum tiles over partitions
        cs = np.asarray(o["out_acc"], np.float32).astype(np.float64).sum(axis=1)
        for s, (l, i, j) in enumerate(SLOTS[c]):
            for t in range(MT):
                col = s * MT + t
                g0 = B * i + 128 * t
                row_s[l][g0 : g0 + 128] += rp[:, col]
                if i == j:
                    row_s[l][g0 : g0 + 128] -= eb[:, col]
                    dij_all[l][g0 : g0 + 128] = dj[:, col]
            if i != j:
                row_s[l][B * j : B * (j + 1)] += cs[ACC_IDX[s]]
    total = 0.0
    for l in range(2):
        s_pair = row_s[l][0::2] + row_s[l][1::2]
        d_ij = dij_all[l][0::2]
        j_val = np.square(np.maximum(np.log(s_pair) - d_ij, 0.0))
        loss = j_val.sum() / P_PAIRS / 2.0
        total += loss if l == 0 else 2.0 * loss
    return np.asarray(total, dtype=np.float32)


def kernel(text_embeddings, shape_embeddings):
    in_maps = _make_in_maps(text_embeddings, shape_embeddings)
    nc = _get_nc()
    res = run_bass_kernel_spmd(nc, in_maps, core_ids=list(range(NCORES)))
    return _finalize(res.results)



# revision 4
# speedup vs baseline: 1.0670x; 1.0670x over previous
"""Trainium2 Bass kernel for the lifted-structure metric loss (nn_Metric_Loss).

Even/odd decomposition: with A = T[0::2], B = T[1::2], C = S (all [2048,1024]),
the two losses need the Gram products P1=A.A', P2=A.B', P3=B.B' (loss T) and
P1, P4=A.C', P5=C.C' (loss Z) -- P1 is shared, and the symmetric products only
need their upper triangles.  In 512-blocks that is 3*10 + 2*16 = 62 block
matmuls (vs 72 for the interleaved formulation).

Per pair p (rows of A / interleaved rows 2p,2p+1):
    S_T[p] = rs1[p] + rowsumE(P2)[p] + colsumE(P2)[p] + rs3[p] - eb_T[p]
where rs* are full symmetric row sums of exp(m + P*) and eb/dij corrections
are O(N d) dot products of the quantized inputs computed on the HOST -- no
on-device mask work at all.

Device (SPMD, identical program, per-core data): 4 "pairs" per core, each
pair = (lhs 512-block, 2 rhs 512-blocks).  16 MMs per (pair, t) quad... per
t: 8 fp8 DoubleRow matmuls into a 2-bank PSUM tile, one [128,1024] Exp
activation with row-sum accumulation, then a 3-add tree for the column-sum
tiles shipped as bf16.  Every core's 4th pair has a diagonal rhs1 whose
column sums are not needed, so the tail after the last matmul is one
activation + one add + one small DMA.  A few dummy matmuls at the head keep
the PE busy (HAM warm-up) while the first input blocks stream in.
"""

import numpy as np
import ml_dtypes

import concourse.mybir as mybir
import concourse.tile as tile
from concourse import bacc
from concourse.bass import ds
from concourse.bass_utils import run_bass_kernel_spmd

N, D_EMB = 4096, 1024
P_ROWS = N // 2           # 2048 rows per matrix A/B/C
NCORES = 8
B = 512                   # block size
KC = D_EMB // 128         # 8 k-chunks
MT = B // 128             # 4 m-tiles per block
NPAIR = 4                 # pairs per core
NSLOT = 3 * NPAIR         # input block slots per core
NCS = 2 * (NPAIR - 1) + 1 # shipped col-sum tiles per core (pair 3: rhs0 only)
NDUMMY = 14               # PE warm-up matmuls (N=128) during the DMA head
MARGIN = 0.5

# The deal: per core, 4 pairs of (prod, L, r0, r1); block = (matrix, idx) with
# matrix 0=A, 1=B, 2=C.  prod: 1/3/5 = symmetric Grams of A/B/C, 2 = A.B',
# 4 = A.C', 0 = duplicated filler (host ignores).  Constraints honored:
#  - every product block covered exactly once (sym blocks in one orientation)
#  - each pair's two rhs blocks share the lhs block and host accumulator
#  - pair 3's rhs1 is always the diagonal block (its col sums are not shipped)
PAIRS = [
 [(2,(0,0),(1,0),(1,1)), (2,(0,0),(1,2),(1,3)), (3,(1,3),(1,0),(1,1)), (3,(1,0),(1,1),(1,0))],
 [(2,(0,1),(1,0),(1,1)), (2,(0,1),(1,2),(1,3)), (2,(0,2),(1,0),(1,1)), (3,(1,1),(1,2),(1,1))],
 [(2,(0,2),(1,2),(1,3)), (2,(0,3),(1,0),(1,1)), (2,(0,3),(1,2),(1,3)), (3,(1,2),(1,0),(1,2))],
 [(1,(0,3),(0,0),(0,1)), (1,(0,0),(0,1),(0,0)), (1,(0,1),(0,2),(0,1)), (1,(0,2),(0,0),(0,2))],
 [(4,(0,0),(2,0),(2,1)), (4,(0,0),(2,2),(2,3)), (5,(2,3),(2,0),(2,1)), (5,(2,0),(2,1),(2,0))],
 [(4,(0,1),(2,0),(2,1)), (4,(0,1),(2,2),(2,3)), (4,(0,2),(2,0),(2,1)), (5,(2,1),(2,2),(2,1))],
 [(4,(0,2),(2,2),(2,3)), (4,(0,3),(2,0),(2,1)), (4,(0,3),(2,2),(2,3)), (5,(2,2),(2,0),(2,2))],
 [(1,(0,3),(0,2),(0,3)), (3,(1,3),(1,2),(1,3)), (0,(1,3),(1,0),(1,1)), (5,(2,3),(2,2),(2,3))],
]

_CACHE = {}


def _build_nc():
    nc = bacc.Bacc(
        "TRN2",
        target_bir_lowering=False,
        debug=False,
        num_devices=NCORES,
        enable_partition_id=False,
        monotonic_sem_count=0,
    )
    f32 = mybir.dt.float32
    bf16 = mybir.dt.bfloat16
    fp8 = mybir.dt.float8e4
    DR = mybir.MatmulPerfMode.DoubleRow
    blk = nc.dram_tensor("blk", [NSLOT, 128, KC, B], fp8, kind="ExternalInput").ap()
    out_main = nc.dram_tensor(
        "out_main", [128, NPAIR * MT], f32, kind="ExternalOutput"
    ).ap()
    out_cs = nc.dram_tensor("out_cs", [NCS, 128, B], bf16, kind="ExternalOutput").ap()

    with tile.TileContext(nc) as tc:
        with (
            tc.tile_pool(name="consts", bufs=1) as consts,
            tc.tile_pool(name="psum", bufs=4, space="PSUM") as psum_pool,
            tc.tile_pool(name="esc", bufs=2) as esc_pool,
            tc.tile_pool(name="stats", bufs=3) as stats,
        ):
            bias_sb = consts.tile([128, 1], f32, tag="bias")
            nc.vector.memset(bias_sb, MARGIN)
            rp_sb = consts.tile([128, NPAIR * MT], f32, tag="rp")
            # warm-up fodder: tiny zeroed fp8 tile for dummy matmuls
            dum = consts.tile([128, 2, 128], fp8, tag="dum")
            nc.vector.memset(dum, 0.0)

            # single resident input tile; pair 0 arrives in k-chunk pairs so
            # its first matmuls start as soon as the first chunks land
            blk_sb = consts.tile([128, NSLOT, KC, B], fp8, tag="blk")
            for kp in range(KC // 2):
                nc.sync.dma_start(
                    out=blk_sb[:, 0:3, 2 * kp : 2 * kp + 2, :],
                    in_=blk[0:3, :, 2 * kp : 2 * kp + 2, :].rearrange(
                        "s p k c -> p s k c"
                    ),
                )
            for pr in range(1, NPAIR):
                nc.sync.dma_start(
                    out=blk_sb[:, 3 * pr : 3 * pr + 3, :, :],
                    in_=blk[3 * pr : 3 * pr + 3].rearrange("s p k c -> p s k c"),
                )

            # pair 0 PSUM quads allocated up front (k-outer loop below)
            psums0 = [
                psum_pool.tile([128, 2, B], f32, tag="ps", name=f"ps0_{t}")
                for t in range(MT)
            ]

            # dummy matmuls: keep the PE busy from the preamble on, so the
            # HAM clock-gate is warm when the real stream starts
            for _ in range(NDUMMY):
                nc.tensor.matmul(
                    psums0[0][:, 0, 0:128], dum, dum,
                    start=True, stop=True, perf_mode=DR,
                )

            # pair 0: k-outer so each k-chunk-pair DMA feeds 8 matmuls
            for k2 in range(KC // 2):
                for t in range(MT):
                    for r in range(2):
                        nc.tensor.matmul(
                            psums0[t][:, r, :],
                            blk_sb[:, 0, 2 * k2 : 2 * k2 + 2, ds(128 * t, 128)],
                            blk_sb[:, 1 + r, 2 * k2 : 2 * k2 + 2, :],
                            start=(k2 == 0), stop=(k2 == KC // 2 - 1),
                            perf_mode=DR,
                        )

            for pr in range(NPAIR):
                esc = esc_pool.tile([128, MT, 2, B], f32, tag="esc")
                for t in range(MT):
                    if pr == 0:
                        ps = psums0[t]
                    else:
                        ps = psum_pool.tile([128, 2, B], f32, tag="ps")
                        for k2 in range(KC // 2):
                            for r in range(2):
                                nc.tensor.matmul(
                                    ps[:, r, :],
                                    blk_sb[:, 3 * pr, 2 * k2 : 2 * k2 + 2,
                                           ds(128 * t, 128)],
                                    blk_sb[:, 3 * pr + 1 + r, 2 * k2 : 2 * k2 + 2, :],
                                    start=(k2 == 0), stop=(k2 == KC // 2 - 1),
                                    perf_mode=DR,
                                )
                    col = MT * pr + t
                    nc.scalar.activation(
                        esc[:, t], ps,
                        mybir.ActivationFunctionType.Exp,
                        bias=bias_sb, scale=1.0,
                        accum_out=rp_sb[:, col : col + 1],
                    )
                # col sums: tree over the 4 m-tiles, shipped bf16; the host
                # reduces over partitions.  Pair 3's rhs1 is diagonal -> only
                # rhs0 is shipped, keeping the post-matmul tail short.
                if pr < NPAIR - 1:
                    a01 = stats.tile([128, 2, B], f32, tag="a01")
                    nc.vector.tensor_add(a01, esc[:, 0], esc[:, 1])
                    a012 = stats.tile([128, 2, B], f32, tag="a012")
                    nc.gpsimd.tensor_add(a012, a01, esc[:, 2])
                    csb = stats.tile([128, 2, B], bf16, tag="csb")
                    nc.vector.tensor_add(csb, a012, esc[:, 3])
                    nc.sync.dma_start(
                        out=out_cs[2 * pr : 2 * pr + 2].rearrange("r p c -> p r c"),
                        in_=csb,
                    )
                else:
                    a01 = stats.tile([128, B], f32, tag="a01l")
                    nc.vector.tensor_add(a01, esc[:, 0, 0, :], esc[:, 1, 0, :])
                    a012 = stats.tile([128, B], f32, tag="a012l")
                    nc.gpsimd.tensor_add(a012, a01, esc[:, 2, 0, :])
                    csb = stats.tile([128, B], bf16, tag="csbl")
                    nc.vector.tensor_add(csb, a012, esc[:, 3, 0, :])
                    nc.sync.dma_start(out=out_cs[NCS - 1], in_=csb)
            nc.sync.dma_start(out=out_main, in_=rp_sb)
    nc.compile()
    return nc


def _get_nc():
    if "nc" not in _CACHE:
        _CACHE["nc"] = _build_nc()
    return _CACHE["nc"]


def _make_in_maps(text_embeddings, shape_embeddings):
    T = np.asarray(text_embeddings, dtype=np.float32)
    S = np.asarray(shape_embeddings, dtype=np.float32)
    fp8 = ml_dtypes.float8_e4m3
    q8 = (T[0::2].astype(fp8), T[1::2].astype(fp8), S.astype(fp8))

    def xg(M8):  # [2048, 1024] -> [128, KC, 2048] transposed-chunk layout
        XT = np.ascontiguousarray(M8.T)
        return XT.reshape(KC, 128, P_ROWS).transpose(1, 0, 2)

    G = [xg(m) for m in q8]
    in_maps = []
    for c in range(NCORES):
        blk = np.empty((NSLOT, 128, KC, B), dtype=fp8)
        for p, (prod, L, r0, r1) in enumerate(PAIRS[c]):
            for si, (m, i) in enumerate((L, r0, r1)):
                blk[3 * p + si] = G[m][:, :, B * i : B * (i + 1)]
        in_maps.append({"blk": blk})
    return in_maps, q8


def _finalize(outs, q8):
    A, Bm, C = (m.astype(np.float64) for m in q8)
    rs = {k: np.zeros(P_ROWS, np.float64) for k in (1, 2, 3, 4, 5)}
    cs = {1: rs[1], 3: rs[3], 5: rs[5],
          2: np.zeros(P_ROWS, np.float64), 4: np.zeros(P_ROWS, np.float64)}
    for c, o in enumerate(outs):
        rp = np.asarray(o["out_main"], np.float64)           # [128, 16]
        ct = np.asarray(o["out_cs"], np.float32).astype(np.float64)  # [7,128,512]
        for p, (prod, L, r0, r1) in enumerate(PAIRS[c]):
            if prod == 0:
                continue
            li = L[1]
            for t in range(MT):
                g0 = B * li + 128 * t
                rs[prod][g0 : g0 + 128] += rp[:, MT * p + t]
            for ri, r in enumerate((r0, r1)):
                if p == NPAIR - 1 and ri == 1:
                    continue                                  # not shipped
                if prod in (1, 3, 5) and r == L:
                    continue                                  # diag: rows only
                colsum = ct[2 * p + ri].sum(axis=0)           # [512]
                cs[prod][B * r[1] : B * (r[1] + 1)] += colsum
    d1 = np.einsum("ij,ij->i", A, A)
    d2 = np.einsum("ij,ij->i", A, Bm)
    d3 = np.einsum("ij,ij->i", Bm, Bm)
    d4 = np.einsum("ij,ij->i", A, C)
    d5 = np.einsum("ij,ij->i", C, C)
    m = MARGIN
    s_t = rs[1] + rs[2] + cs[2] + rs[3] - (
        np.exp(m + d1) + 2.0 * np.exp(m + d2) + np.exp(m + d3))
    s_z = rs[1] + rs[4] + cs[4] + rs[5] - (
        np.exp(m + d1) + 2.0 * np.exp(m + d4) + np.exp(m + d5))
    j_t = np.square(np.maximum(np.log(s_t) - d2, 0.0))
    j_z = np.square(np.maximum(np.log(s_z) - d4, 0.0))
    total = j_t.mean() / 2.0 + 2.0 * (j_z.mean() / 2.0)
    return np.asarray(total, dtype=np.float32)


def kernel(text_embeddings, shape_embeddings):
    in_maps, q8 = _make_in_maps(text_embeddings, shape_embeddings)
    nc = _get_nc()
    res = run_bass_kernel_spmd(nc, in_maps, core_ids=list(range(NCORES)))
    return _finalize(res.results, q8)


# revision 12
# speedup vs baseline: 1.1010x; 1.0319x over previous
"""Trainium2 Bass kernel for the lifted-structure metric loss (nn_Metric_Loss).

Even/odd decomposition: with A = T[0::2], B = T[1::2], C = S (all [2048,1024]),
the two losses need the Gram products P1=A.A', P2=A.B', P3=B.B' (loss T) and
P1, P4=A.C', P5=C.C' (loss Z) -- P1 is shared, and the symmetric products only
need their upper triangles.  In 512-blocks that is 3*10 + 2*16 = 62 block
matmuls (vs 72 for the interleaved formulation).

Per pair p (rows of A / interleaved rows 2p,2p+1):
    S_T[p] = rs1[p] + rowsumE(P2)[p] + colsumE(P2)[p] + rs3[p] - eb_T[p]
where rs* are full symmetric row sums of exp(m + P*) and eb/dij corrections
are O(N d) dot products of the quantized inputs computed on the HOST -- no
on-device mask work at all.

Device (SPMD, identical program, per-core data): 4 "pairs" per core, each
pair = (lhs 512-block, 2 rhs 512-blocks).  16 MMs per (pair, t) quad... per
t: 8 fp8 DoubleRow matmuls into a 2-bank PSUM tile, one [128,1024] Exp
activation with row-sum accumulation, then a 3-add tree for the column-sum
tiles shipped as bf16.  Every core's 4th pair has a diagonal rhs1 whose
column sums are not needed, so the tail after the last matmul is one
activation + one add + one small DMA.  A few dummy matmuls at the head keep
the PE busy (HAM warm-up) while the first input blocks stream in.
"""

import numpy as np
import ml_dtypes

import concourse.mybir as mybir
import concourse.tile as tile
from concourse import bacc
from concourse.bass import ds
from concourse.bass_utils import run_bass_kernel_spmd

N, D_EMB = 4096, 1024
P_ROWS = N // 2           # 2048 rows per matrix A/B/C
NCORES = 8
B = 512                   # block size
KC = D_EMB // 128         # 8 k-chunks
MT = B // 128             # 4 m-tiles per block
NPAIR = 4                 # pairs per core
NSLOT = 3 * NPAIR         # input block slots per core
NCS = 2 * (NPAIR - 1) + 1 # shipped col-sum tiles per core (pair 3: rhs0 only)
NDUMMY = 28               # PE warm-up matmuls (N=128) during the DMA head
MARGIN = 0.5

# The deal: per core, 4 pairs of (prod, L, r0, r1); block = (matrix, idx) with
# matrix 0=A, 1=B, 2=C.  prod: 1/3/5 = symmetric Grams of A/B/C, 2 = A.B',
# 4 = A.C', 0 = duplicated filler (host ignores).  Constraints honored:
#  - every product block covered exactly once (sym blocks in one orientation)
#  - each pair's two rhs blocks share the lhs block and host accumulator
#  - pair 3's rhs1 is always the diagonal block (its col sums are not shipped)
PAIRS = [
 [(2,(0,0),(1,0),(1,1)), (2,(0,0),(1,2),(1,3)), (3,(1,3),(1,0),(1,1)), (3,(1,0),(1,1),(1,0))],
 [(2,(0,1),(1,0),(1,1)), (2,(0,1),(1,2),(1,3)), (2,(0,2),(1,0),(1,1)), (3,(1,1),(1,2),(1,1))],
 [(2,(0,2),(1,2),(1,3)), (2,(0,3),(1,0),(1,1)), (2,(0,3),(1,2),(1,3)), (3,(1,2),(1,0),(1,2))],
 [(1,(0,3),(0,0),(0,1)), (1,(0,0),(0,1),(0,0)), (1,(0,1),(0,2),(0,1)), (1,(0,2),(0,0),(0,2))],
 [(4,(0,0),(2,0),(2,1)), (4,(0,0),(2,2),(2,3)), (5,(2,3),(2,0),(2,1)), (5,(2,0),(2,1),(2,0))],
 [(4,(0,1),(2,0),(2,1)), (4,(0,1),(2,2),(2,3)), (4,(0,2),(2,0),(2,1)), (5,(2,1),(2,2),(2,1))],
 [(4,(0,2),(2,2),(2,3)), (4,(0,3),(2,0),(2,1)), (4,(0,3),(2,2),(2,3)), (5,(2,2),(2,0),(2,2))],
 [(1,(0,3),(0,2),(0,3)), (3,(1,3),(1,2),(1,3)), (0,(1,3),(1,0),(1,1)), (5,(2,3),(2,2),(2,3))],
]

_CACHE = {}


def _build_nc():
    nc = bacc.Bacc(
        "TRN2",
        target_bir_lowering=False,
        debug=False,
        num_devices=NCORES,
        enable_partition_id=False,
        monotonic_sem_count=0,
    )
    f32 = mybir.dt.float32
    bf16 = mybir.dt.bfloat16
    fp8 = mybir.dt.float8e4
    DR = mybir.MatmulPerfMode.DoubleRow
    blk = nc.dram_tensor("blk", [NSLOT, 128, KC, B], fp8, kind="ExternalInput").ap()
    out_main = nc.dram_tensor(
        "out_main", [128, NPAIR * MT + 1], f32, kind="ExternalOutput"
    ).ap()
    out_cs = nc.dram_tensor("out_cs", [NCS, 128, B], bf16, kind="ExternalOutput").ap()

    with tile.TileContext(nc) as tc:
        with (
            tc.tile_pool(name="consts", bufs=1) as consts,
            tc.tile_pool(name="psum", bufs=4, space="PSUM") as psum_pool,
            tc.tile_pool(name="esc", bufs=2) as esc_pool,
            tc.tile_pool(name="stats", bufs=3) as stats,
        ):
            bias_sb = consts.tile([128, 1], f32, tag="bias")
            nc.vector.memset(bias_sb, MARGIN)
            rp_sb = consts.tile([128, NPAIR * MT + 1], f32, tag="rp")
            # warm-up fodder: tiny zeroed fp8 tile for dummy matmuls
            dum = consts.tile([128, 2, 128], fp8, tag="dum")
            nc.vector.memset(dum, 0.0)

            # single resident input tile; pair 0 arrives in k-chunk pairs so
            # its first matmuls start as soon as the first chunks land
            blk_sb = consts.tile([128, NSLOT, KC, B], fp8, tag="blk")
            for kp in range(KC // 2):
                nc.sync.dma_start(
                    out=blk_sb[:, 0:3, 2 * kp : 2 * kp + 2, :],
                    in_=blk[0:3, :, 2 * kp : 2 * kp + 2, :].rearrange(
                        "s p k c -> p s k c"
                    ),
                )
            # pair 1 in k-halves (its matmuls start before its full data
            # could land); pairs 2-3 whole
            for kh in range(2):
                nc.sync.dma_start(
                    out=blk_sb[:, 3:6, 4 * kh : 4 * kh + 4, :],
                    in_=blk[3:6, :, 4 * kh : 4 * kh + 4, :].rearrange(
                        "s p k c -> p s k c"
                    ),
                )
            for pr in range(2, NPAIR):
                nc.sync.dma_start(
                    out=blk_sb[:, 3 * pr : 3 * pr + 3, :, :],
                    in_=blk[3 * pr : 3 * pr + 3].rearrange("s p k c -> p s k c"),
                )

            # pair 0 PSUM quads allocated up front (k-outer loop below)
            psums0 = [
                psum_pool.tile([128, 2, B], f32, tag="ps", name=f"ps0_{t}")
                for t in range(MT)
            ]

            # dummy matmuls: keep the PE busy from the preamble on, so the
            # HAM clock-gate is warm when the real stream starts
            for _ in range(NDUMMY):
                nc.tensor.matmul(
                    psums0[0][:, 0, 0:128], dum, dum,
                    start=True, stop=True, perf_mode=DR,
                )

            # pair 0: k-outer so each k-chunk-pair DMA feeds 8 matmuls
            for k2 in range(KC // 2):
                for t in range(MT):
                    for r in range(2):
                        nc.tensor.matmul(
                            psums0[t][:, r, :],
                            blk_sb[:, 0, 2 * k2 : 2 * k2 + 2, ds(128 * t, 128)],
                            blk_sb[:, 1 + r, 2 * k2 : 2 * k2 + 2, :],
                            start=(k2 == 0), stop=(k2 == KC // 2 - 1),
                            perf_mode=DR,
                        )

            for pr in range(NPAIR):
                esc = esc_pool.tile([128, MT, 2, B], f32, tag="esc")
                for t in range(MT):
                    if pr == 0:
                        ps = psums0[t]
                    else:
                        ps = psum_pool.tile([128, 2, B], f32, tag="ps")
                        for k2 in range(KC // 2):
                            for r in range(2):
                                nc.tensor.matmul(
                                    ps[:, r, :],
                                    blk_sb[:, 3 * pr, 2 * k2 : 2 * k2 + 2,
                                           ds(128 * t, 128)],
                                    blk_sb[:, 3 * pr + 1 + r, 2 * k2 : 2 * k2 + 2, :],
                                    start=(k2 == 0), stop=(k2 == KC // 2 - 1),
                                    perf_mode=DR,
                                )
                    col = MT * pr + t
                    if pr == NPAIR - 1 and t == MT - 1:
                        # tail: only rhs0's exp feeds the last col-sum add;
                        # split the activation so rhs1 (diag, accum-only)
                        # runs after and off the critical path
                        nc.scalar.activation(
                            esc[:, t, 0], ps[:, 0],
                            mybir.ActivationFunctionType.Exp,
                            bias=bias_sb, scale=1.0,
                            accum_out=rp_sb[:, col : col + 1],
                        )
                        nc.scalar.activation(
                            esc[:, t, 1], ps[:, 1],
                            mybir.ActivationFunctionType.Exp,
                            bias=bias_sb, scale=1.0,
                            accum_out=rp_sb[:, col + 1 : col + 2],
                        )
                    else:
                        nc.scalar.activation(
                            esc[:, t], ps,
                            mybir.ActivationFunctionType.Exp,
                            bias=bias_sb, scale=1.0,
                            accum_out=rp_sb[:, col : col + 1],
                        )
                # col sums: tree over the 4 m-tiles, shipped bf16; the host
                # reduces over partitions.  Pair 3's rhs1 is diagonal -> only
                # rhs0 is shipped, keeping the post-matmul tail short.
                if pr < NPAIR - 1:
                    a01 = stats.tile([128, 2, B], f32, tag="a01")
                    nc.vector.tensor_add(a01, esc[:, 0], esc[:, 1])
                    a012 = stats.tile([128, 2, B], f32, tag="a012")
                    nc.gpsimd.tensor_add(a012, a01, esc[:, 2])
                    csb = stats.tile([128, 2, B], bf16, tag="csb")
                    nc.vector.tensor_add(csb, a012, esc[:, 3])
                    nc.sync.dma_start(
                        out=out_cs[2 * pr : 2 * pr + 2].rearrange("r p c -> p r c"),
                        in_=csb,
                    )
                else:
                    a01 = stats.tile([128, B], f32, tag="a01l")
                    nc.vector.tensor_add(a01, esc[:, 0, 0, :], esc[:, 1, 0, :])
                    a012 = stats.tile([128, B], f32, tag="a012l")
                    nc.gpsimd.tensor_add(a012, a01, esc[:, 2, 0, :])
                    csb = stats.tile([128, B], bf16, tag="csbl")
                    nc.vector.tensor_add(csb, a012, esc[:, 3, 0, :])
                    nc.sync.dma_start(out=out_cs[NCS - 1], in_=csb)
            nc.sync.dma_start(out=out_main, in_=rp_sb)
    nc.compile()
    return nc


def _get_nc():
    if "nc" not in _CACHE:
        _CACHE["nc"] = _build_nc()
    return _CACHE["nc"]


def _make_in_maps(text_embeddings, shape_embeddings):
    T = np.asarray(text_embeddings, dtype=np.float32)
    S = np.asarray(shape_embeddings, dtype=np.float32)
    fp8 = ml_dtypes.float8_e4m3
    q8 = (T[0::2].astype(fp8), T[1::2].astype(fp8), S.astype(fp8))

    def xg(M8):  # [2048, 1024] -> [128, KC, 2048] transposed-chunk layout
        XT = np.ascontiguousarray(M8.T)
        return XT.reshape(KC, 128, P_ROWS).transpose(1, 0, 2)

    G = [xg(m) for m in q8]
    in_maps = []
    for c in range(NCORES):
        blk = np.empty((NSLOT, 128, KC, B), dtype=fp8)
        for p, (prod, L, r0, r1) in enumerate(PAIRS[c]):
            for si, (m, i) in enumerate((L, r0, r1)):
                blk[3 * p + si] = G[m][:, :, B * i : B * (i + 1)]
        in_maps.append({"blk": blk})
    return in_maps, q8


def _finalize(outs, q8):
    A, Bm, C = (m.astype(np.float64) for m in q8)
    rs = {k: np.zeros(P_ROWS, np.float64) for k in (1, 2, 3, 4, 5)}
    cs = {1: rs[1], 3: rs[3], 5: rs[5],
          2: np.zeros(P_ROWS, np.float64), 4: np.zeros(P_ROWS, np.float64)}
    for c, o in enumerate(outs):
        rp = np.asarray(o["out_main"], np.float64)           # [128, 16]
        ct = np.asarray(o["out_cs"], np.float32).astype(np.float64)  # [7,128,512]
        for p, (prod, L, r0, r1) in enumerate(PAIRS[c]):
            if prod == 0:
                continue
            li = L[1]
            for t in range(MT):
                g0 = B * li + 128 * t
                rs[prod][g0 : g0 + 128] += rp[:, MT * p + t]
                if p == NPAIR - 1 and t == MT - 1:
                    rs[prod][g0 : g0 + 128] += rp[:, MT * p + t + 1]
            for ri, r in enumerate((r0, r1)):
                if p == NPAIR - 1 and ri == 1:
                    continue                                  # not shipped
                if prod in (1, 3, 5) and r == L:
                    continue                                  # diag: rows only
                colsum = ct[2 * p + ri].sum(axis=0)           # [512]
                cs[prod][B * r[1] : B * (r[1] + 1)] += colsum
    d1 = np.einsum("ij,ij->i", A, A)
    d2 = np.einsum("ij,ij->i", A, Bm)
    d3 = np.einsum("ij,ij->i", Bm, Bm)
    d4 = np.einsum("ij,ij->i", A, C)
    d5 = np.einsum("ij,ij->i", C, C)
    m = MARGIN
    s_t = rs[1] + rs[2] + cs[2] + rs[3] - (
        np.exp(m + d1) + 2.0 * np.exp(m + d2) + np.exp(m + d3))
    s_z = rs[1] + rs[4] + cs[4] + rs[5] - (
        np.exp(m + d1) + 2.0 * np.exp(m + d4) + np.exp(m + d5))
    j_t = np.square(np.maximum(np.log(s_t) - d2, 0.0))
    j_z = np.square(np.maximum(np.log(s_z) - d4, 0.0))
    total = j_t.mean() / 2.0 + 2.0 * (j_z.mean() / 2.0)
    return np.asarray(total, dtype=np.float32)


def kernel(text_embeddings, shape_embeddings):
    in_maps, q8 = _make_in_maps(text_embeddings, shape_embeddings)
    nc = _get_nc()
    res = run_bass_kernel_spmd(nc, in_maps, core_ids=list(range(NCORES)))
    return _finalize(res.results, q8)
